# revision 36
# baseline (speedup 1.0000x reference)
# Self-contained TRN2 Bass kernel for nn_Attention_NMT (B=32,S=T=100,H=E=512,V=32000).
# SPMD over 8 NeuronCores, batch-parallel (4 batch rows per core).
# v2: sb-major attention (scores via DVE accumulate), tanh-domain LSTM cell
# (no act-table swaps), bias folded into cls2 copies.
import sys
for _p in ("/opt/trn_rl_repo",):
    if _p not in sys.path:
        sys.path.insert(0, _p)
import numpy as np
import ml_dtypes
BF = ml_dtypes.bfloat16
import concourse.bass as bass
import concourse.bacc as bacc
import concourse.tile as tile
from concourse import mybir
from concourse.bass import ds
from concourse.masks import make_identity

F32 = mybir.dt.float32
BF16 = mybir.dt.bfloat16
I32 = mybir.dt.int32
AF = mybir.ActivationFunctionType
ALU = mybir.AluOpType

E = 512; H = 512; G = 2048; A = 1536; Bl = 4
EK = E // 128; HK = H // 128; H2K = 2 * H // 128; AJ = A // 128
GN = G // 512; AN = A // 512


def build_nc(S=100, T=100, V=32000, num_devices=8, unroll=4, npos=768):
    NQ = (S + 31) // 32          # source quarters / token tiles
    NQT = (T + 31) // 32
    SP = 32 * NQ                 # padded
    TP = 32 * NQT

    nc = bacc.Bacc("TRN2", target_bir_lowering=False, debug=False,
                   num_devices=num_devices)

    def din(name, shape, dt=F32):
        return nc.dram_tensor(name, shape, dt, kind="ExternalInput")

    src_emb = din("src_emb", [V, E]); tgt_emb = din("tgt_emb", [V, E])
    idx_src = din("idx_src", [128, NQ], I32)
    idx_tgt = din("idx_tgt", [128, NQT], I32)
    WihfT = din("WihfT", [E, G], BF16); WihbT = din("WihbT", [E, G], BF16)
    WiheT = din("WiheT", [E, G], BF16)
    WhhfT = din("WhhfT", [H, G], BF16); WhhbT = din("WhhbT", [H, G], BF16)
    WqWhhT = din("WqWhhT", [H, A + G], BF16)
    WihcT = din("WihcT", [2 * H, G], BF16)
    A1eT = din("A1eT", [2 * H, A], BF16)
    biasf = din("biasf", [1, G], BF16); biasb = din("biasb", [1, G], BF16)
    biasd = din("biasd", [1, G], BF16); att1b = din("att1b", [1, A], BF16)
    cls1WT = din("cls1WT", [G, 1024], BF16); cls1b = din("cls1b", [1, 1024], BF16)
    cls2WT = din("cls2WT", [1024, V], BF16)
    cls2bR = din("cls2bR", [128, V], BF16)
    mask4 = din("mask4", [128, Bl])
    mask4v = din("mask4v", [128, Bl])
    mask4F = din("mask4F", [128, (S + 31) // 32 * Bl])
    rep4 = din("rep4", [Bl, 128])

    logits = nc.dram_tensor("logits", [T, Bl, V], BF16, kind="ExternalOutput")

    # DRAM scratch
    Xf_d = nc.dram_tensor("Xf_d", [S, Bl, G], BF16)
    Xb_d = nc.dram_tensor("Xb_d", [S, Bl, G], BF16)
    Xd_d = nc.dram_tensor("Xd_d", [T, Bl, G], BF16)
    of_d = nc.dram_tensor("of_d", [S, Bl, H], BF16)
    ob_d = nc.dram_tensor("ob_d", [S, Bl, H], BF16)
    ctx_d = nc.dram_tensor("ctx_d", [T, Bl, 2 * H], BF16)
    dec_d = nc.dram_tensor("dec_d", [T, Bl, H], BF16)

    with tile.TileContext(nc) as tc:
        from contextlib import ExitStack
        _stack = ExitStack()
        persist = _stack.enter_context(tc.tile_pool(name="persist", bufs=1))

        # ---- constants ----
        I128 = persist.tile([128, 128], F32)
        make_identity(nc, I128[:, :])
        ones = persist.tile([1, 512], BF16)
        nc.vector.memset(ones[:, :], 1.0)
        I128b = persist.tile([128, 128], BF16)
        nc.vector.tensor_copy(I128b[:, :], I128[:, :])
        mask4_s = persist.tile([128, Bl], F32)
        nc.gpsimd.dma_start(out=mask4_s[:, :], in_=mask4[:, :])
        rep4_s = persist.tile([Bl, 128], F32)
        nc.gpsimd.dma_start(out=rep4_s[:, :], in_=rep4[:, :])
        rep4_b = persist.tile([Bl, 128], BF16)
        nc.vector.tensor_copy(rep4_b[:, :], rep4_s[:, :])
        mask4v_s = persist.tile([128, Bl], F32)
        nc.gpsimd.dma_start(out=mask4v_s[:, :], in_=mask4v[:, :])
        mask4F_s = persist.tile([128, NQ, Bl], F32)
        nc.gpsimd.dma_start(out=mask4F_s[:, :, :],
                            in_=mask4F[:, :].rearrange("p (q b) -> p q b", b=Bl))
        I4b = persist.tile([Bl, Bl], BF16)
        nc.vector.tensor_copy(I4b[:, :], I128[:Bl, :Bl])
        att1b_s = persist.tile([1, A], BF16)
        nc.gpsimd.dma_start(out=att1b_s[:, :], in_=att1b[:, :])

        # persistent activations
        tembT = persist.tile([128, EK, 128 * NQT], BF16)
        h_f = persist.tile([Bl, H], F32); c_f = persist.tile([Bl, H], F32)
        h_b = persist.tile([Bl, H], F32); c_b = persist.tile([Bl, H], F32)
        for t_ in (h_f, c_f, h_b, c_b):
            nc.vector.memset(t_[:, :], 0.0)

        # ================= PHASE 0: embeddings + X GEMMs =================
        with tc.tile_pool(name="ph0", bufs=1) as ph0, \
             tc.tile_pool(name="ph0ps", bufs=2, space="PSUM") as ph0ps, \
             tc.tile_pool(name="ph0st", bufs=2) as ph0st:
            idxs = ph0.tile([128, NQ], I32)
            nc.gpsimd.dma_start(out=idxs[:, :], in_=idx_src[:, :])
            idxt = ph0.tile([128, NQT], I32)
            nc.gpsimd.dma_start(out=idxt[:, :], in_=idx_tgt[:, :])
            xQ = ph0.tile([128, NQ, E], F32)
            tembQ = ph0.tile([128, NQT, E], F32)
            for q in range(NQ):
                nc.gpsimd.indirect_dma_start(
                    out=xQ[:, q, :], out_offset=None, in_=src_emb[:, :],
                    in_offset=bass.IndirectOffsetOnAxis(ap=idxs[:, q:q + 1], axis=0))
            for q in range(NQT):
                nc.gpsimd.indirect_dma_start(
                    out=tembQ[:, q, :], out_offset=None, in_=tgt_emb[:, :],
                    in_offset=bass.IndirectOffsetOnAxis(ap=idxt[:, q:q + 1], axis=0))

            # transpose xQ/tembQ -> xT/tembT  (feature-major, token cols)
            xT = ph0.tile([128, EK, 128 * NQ], BF16)
            for q in range(NQ):
                pT = ph0ps.tile([128, EK, 128], F32, space="PSUM")
                for kc in range(EK):
                    nc.tensor.transpose(out=pT[:, kc, :],
                                        in_=xQ[:, q, 128 * kc:128 * (kc + 1)],
                                        identity=I128[:, :])
                nc.vector.tensor_copy(xT[:, :, 128 * q:128 * (q + 1)], pT[:, :, :])
            for q in range(NQT):
                pT = ph0ps.tile([128, EK, 128], F32, space="PSUM")
                for kc in range(EK):
                    nc.tensor.transpose(out=pT[:, kc, :],
                                        in_=tembQ[:, q, 128 * kc:128 * (kc + 1)],
                                        identity=I128[:, :])
                nc.vector.tensor_copy(tembT[:, :, 128 * q:128 * (q + 1)], pT[:, :, :])

            # X GEMMs -> DRAM   (token-stationary, stream W)
            def x_gemm(wT_dram, bias_dram, lhsT_tile, nQ, S_, out_dram):
                Ws = ph0.tile([128, EK, G], BF16, tag="ws_" + wT_dram.name)
                nc.gpsimd.dma_start(
                    out=Ws[:, 0:EK // 2, :],
                    in_=wT_dram[0:E // 2, :].rearrange("(k p) g -> p k g",
                                                       p=128))
                nc.scalar.dma_start(
                    out=Ws[:, EK // 2:EK, :],
                    in_=wT_dram[E // 2:E, :].rearrange("(k p) g -> p k g",
                                                       p=128))
                bia = ph0.tile([1, G], BF16, tag="bia_" + wT_dram.name)
                nc.gpsimd.dma_start(out=bia[:, :], in_=bias_dram[:, :])
                for Tt in range(nQ):
                    rows = min(32, S_ - 32 * Tt) * Bl
                    stage = ph0st.tile([128, G], BF16, tag="xstage")
                    for n in range(GN):
                        ps = ph0ps.tile([128, 512], F32, space="PSUM", tag="xps")
                        for kc in range(EK):
                            nc.tensor.matmul(
                                out=ps[:, :],
                                lhsT=(lhsT_tile[:, kc, 128 * Tt:128 * (Tt + 1)]),
                                rhs=(Ws[:, kc, 512 * n:512 * (n + 1)]),
                                start=(kc == 0), stop=False)
                        nc.tensor.matmul(
                            out=ps[:, :], lhsT=(ones[:1, :128]),
                            rhs=(bia[:, 512 * n:512 * (n + 1)]),
                            start=False, stop=True)
                        nc.scalar.copy(stage[:, 512 * n:512 * (n + 1)], ps[:, :])
                    nc.sync.dma_start(
                        out=out_dram[32 * Tt:32 * Tt + rows // Bl, :,
                                     :].flatten_outer_dims(),
                        in_=stage[:rows, :])
            x_gemm(WihfT, biasf, xT, NQ, S, Xf_d)
            x_gemm(WihbT, biasb, xT, NQ, S, Xb_d)
            x_gemm(WiheT, biasd, tembT, NQT, T, Xd_d)

        # ================= PHASE 1: encoder =================
        with tc.tile_pool(name="ph1w", bufs=1) as ph1w, \
             tc.tile_pool(name="xst", bufs=3) as xstp, \
             tc.tile_pool(name="cell", bufs=1) as cellp, \
             tc.tile_pool(name="hT", bufs=2) as hTp, \
             tc.tile_pool(name="encg", bufs=1, space="PSUM") as encg:
            Whhf_s = ph1w.tile([128, HK, G], BF16)
            nc.gpsimd.dma_start(out=Whhf_s[:, 0:HK // 2, :],
                in_=WhhfT[0:H // 2, :].rearrange("(k p) g -> p k g", p=128))
            nc.scalar.dma_start(out=Whhf_s[:, HK // 2:HK, :],
                in_=WhhfT[H // 2:H, :].rearrange("(k p) g -> p k g", p=128))
            Whhb_s = ph1w.tile([128, HK, G], BF16)
            nc.gpsimd.dma_start(out=Whhb_s[:, 0:HK // 2, :],
                in_=WhhbT[0:H // 2, :].rearrange("(k p) g -> p k g", p=128))
            nc.scalar.dma_start(out=Whhb_s[:, HK // 2:HK, :],
                in_=WhhbT[H // 2:H, :].rearrange("(k p) g -> p k g", p=128))

            def lstm_part1(h, c, Whh_s, X_d, s_expr, tagp):
                xst = xstp.tile([Bl, G], BF16, tag="xst" + tagp)
                nc.sync.dma_start(
                    out=xst[:, :], in_=X_d[ds(s_expr, 1)].flatten_outer_dims())
                gfull = encg.tile([128, G], F32, space="PSUM", tag="g" + tagp)
                for kc in range(HK):
                    nc.tensor.transpose(out=gfull[:, Bl * kc:Bl * (kc + 1)],
                                        in_=h[:, 128 * kc:128 * (kc + 1)],
                                        identity=I128[:Bl, :Bl])
                hTs = hTp.tile([128, HK, Bl], BF16, tag="hT" + tagp)
                nc.vector.tensor_copy(
                    hTs[:, :, :],
                    gfull[:, 0:HK * Bl].rearrange("p (k b) -> p k b", b=Bl))
                gates = gfull[0:Bl, :]
                # f-slice first so the cell chain starts early, then i, g, o
                for n in (1, 0, 2, 3):
                    gsl = gates[:, 512 * n:512 * (n + 1)]
                    for kc in range(HK):
                        nc.tensor.matmul(out=gsl, lhsT=hTs[:, kc, :],
                                         rhs=Whh_s[:, kc, 512 * n:512 * (n + 1)],
                                         start=(kc == 0), stop=False)
                    nc.tensor.matmul(out=gsl, lhsT=I4b[:, :],
                                     rhs=xst[:, 512 * n:512 * (n + 1)],
                                     start=False, stop=True)
                sf = cellp.tile([Bl, H], F32, tag="sf" + tagp)
                nc.scalar.activation(out=sf[:, :], in_=gates[:, 512:1024],
                                     func=AF.Sigmoid)
                si = cellp.tile([Bl, H], F32, tag="si" + tagp)
                nc.scalar.activation(out=si[:, :], in_=gates[:, 0:512],
                                     func=AF.Sigmoid)
                tg = cellp.tile([Bl, H], F32, tag="tg" + tagp)
                nc.scalar.activation(out=tg[:, :], in_=gates[:, 1024:1536],
                                     func=AF.Tanh)
                so = cellp.tile([Bl, H], F32, tag="so" + tagp)
                nc.scalar.activation(out=so[:, :], in_=gates[:, 1536:2048],
                                     func=AF.Sigmoid)
                p1 = cellp.tile([Bl, H], F32, tag="p1" + tagp)
                nc.vector.tensor_mul(p1[:, :], si[:, :], tg[:, :])
                p2 = cellp.tile([Bl, H], F32, tag="p2" + tagp)
                nc.vector.tensor_mul(p2[:, :], sf[:, :], c[:, :])
                nc.vector.tensor_add(c[:, :], p1[:, :], p2[:, :])
                return so

            def lstm_part2(h, c, so, s_expr, store_d, tagp):
                tcn = cellp.tile([Bl, H], F32, tag="tc" + tagp)
                nc.scalar.activation(out=tcn[:, :], in_=c[:, :], func=AF.Tanh)
                nc.vector.tensor_mul(h[:, :], so[:, :], tcn[:, :])
                if store_d is not None:
                    hbf = cellp.tile([Bl, H], BF16, tag="hbf" + tagp)
                    nc.vector.tensor_copy(hbf[:, :], h[:, :])
                    nc.sync.dma_start(
                        out=store_d[ds(s_expr, 1)].flatten_outer_dims(),
                        in_=hbf[:, :])

            with tc.For_i(0, S, 10) as i0:
                for u_ in range(10):
                    so_f = lstm_part1(h_f, c_f, Whhf_s, Xf_d, i0 + u_, "f")
                    so_b = lstm_part1(h_b, c_b, Whhb_s, Xb_d,
                                      (S - 1 - u_) - i0, "b")
                    lstm_part2(h_f, c_f, so_f, i0 + u_, of_d, "f")
                    lstm_part2(h_b, c_b, so_b, (S - 1 - u_) - i0, ob_d, "b")

        # decoder runs in the 2x domain: H = 2h, C = 2c
        nc.vector.tensor_scalar_mul(h_f[:, :], h_f[:, :], 2.0)
        nc.vector.tensor_scalar_mul(c_f[:, :], c_f[:, :], 2.0)

        # ============ PHASE 2: assemble enc tiles + transposes ============
        scopeB_cm = tc.tile_pool(name="scopeB", bufs=1)
        scopeB = scopeB_cm.__enter__()
        ofQ = scopeB.tile([128, NQ, H], BF16)
        obQ = scopeB.tile([128, NQ, H], BF16)
        u_sb = scopeB.tile([128, NQ, A], BF16)
        encWcQ = scopeB.tile([128, NQ, G], BF16)
        ph23_cm = tc.tile_pool(name="ph23", bufs=1)
        ph23 = ph23_cm.__enter__()
        encT = ph23.tile([128, H2K, 128 * NQ], BF16)
        with tc.tile_pool(name="ph2ps", bufs=2, space="PSUM") as ph2ps:
            nc.vector.memset(ofQ[:, :, :], 0.0)
            nc.vector.memset(obQ[:, :, :], 0.0)
            for q in range(NQ):
                rows = min(32, S - 32 * q) * Bl
                nc.sync.dma_start(
                    out=ofQ[:rows, q, :],
                    in_=of_d[32 * q:32 * q + rows // Bl].flatten_outer_dims())
                nc.sync.dma_start(
                    out=obQ[:rows, q, :],
                    in_=ob_d[32 * q:32 * q + rows // Bl].flatten_outer_dims())
            for q in range(NQ):
                pT = ph2ps.tile([128, HK, 128], BF16, space="PSUM")
                for kc in range(HK):
                    nc.tensor.transpose(out=pT[:, kc, :],
                                        in_=ofQ[:, q, 128 * kc:128 * (kc + 1)],
                                        identity=I128b[:, :])
                nc.vector.tensor_copy(encT[:, 0:HK, 128 * q:128 * (q + 1)],
                                      pT[:, :, :])
                pT2 = ph2ps.tile([128, HK, 128], BF16, space="PSUM")
                for kc in range(HK):
                    nc.tensor.transpose(out=pT2[:, kc, :],
                                        in_=obQ[:, q, 128 * kc:128 * (kc + 1)],
                                        identity=I128b[:, :])
                nc.vector.tensor_copy(encT[:, HK:H2K, 128 * q:128 * (q + 1)],
                                      pT2[:, :, :])

        # ============ PHASE 3: u GEMM (sb-major) + encWcQ GEMM ============
        with tc.tile_pool(name="ph3", bufs=1) as ph3, \
             tc.tile_pool(name="ph3st", bufs=3) as ph3st, \
             tc.tile_pool(name="ph3ps", bufs=2, space="PSUM") as ph3ps:
            A1e_s = ph3.tile([128, H2K, A], BF16)
            nc.gpsimd.dma_start(out=A1e_s[:, 0:H2K // 2, :],
                in_=A1eT[0:H, :].rearrange("(k p) a -> p k a", p=128))
            nc.scalar.dma_start(out=A1e_s[:, H2K // 2:H2K, :],
                in_=A1eT[H:2 * H, :].rearrange("(k p) a -> p k a", p=128))
            # u[sb, a] = enc[sb, :] @ A1e + b   (sb-major output)
            for cq in range(NQ):
                for n in range(AN):
                    ps = ph3ps.tile([128, 512], F32, space="PSUM", tag="ups")
                    for kc in range(H2K):
                        nc.tensor.matmul(
                            out=ps[:, :],
                            lhsT=(encT[:, kc, 128 * cq:128 * (cq + 1)]),
                            rhs=(A1e_s[:, kc, 512 * n:512 * (n + 1)]),
                            start=(kc == 0), stop=False)
                    nc.tensor.matmul(
                        out=ps[:, :], lhsT=(ones[:1, :128]),
                        rhs=(att1b_s[:, 512 * n:512 * (n + 1)]),
                        start=False, stop=True)
                    nc.scalar.copy(u_sb[:, cq, 512 * n:512 * (n + 1)], ps[:, :])
            # encWcQ: token-stationary, stream WihcT chunks from DRAM
            for n in range(GN):
                Wc_n = ph3st.tile([128, H2K, 512], BF16, tag="wcn")
                nc.gpsimd.dma_start(out=Wc_n[:, 0:H2K // 2, :],
                    in_=WihcT[0:H, 512 * n:512 * (n + 1)].rearrange(
                        "(k p) g -> p k g", p=128))
                nc.scalar.dma_start(out=Wc_n[:, H2K // 2:H2K, :],
                    in_=WihcT[H:2 * H, 512 * n:512 * (n + 1)].rearrange(
                        "(k p) g -> p k g", p=128))
                for Tt in range(NQ):
                    ps = ph3ps.tile([128, 512], F32, space="PSUM", tag="wcps")
                    for kc in range(H2K):
                        nc.tensor.matmul(
                            out=ps[:, :],
                            lhsT=(encT[:, kc, 128 * Tt:128 * (Tt + 1)]),
                            rhs=(Wc_n[:, kc, :]), start=(kc == 0),
                            stop=(kc == H2K - 1))
                    nc.scalar.copy(encWcQ[:, Tt, 512 * n:512 * (n + 1)], ps[:, :])

        ph23_cm.__exit__(None, None, None)

        # ================= PHASE 4: decoder =================
        h = h_f; c = c_f   # decoder init = final fwd encoder state (2x domain)
        nneg = A - npos
        with tc.tile_pool(name="ph4w", bufs=1) as ph4w, \
             tc.tile_pool(name="dxst", bufs=3) as dxstp, \
             tc.tile_pool(name="dcell", bufs=1) as dcellp, \
             tc.tile_pool(name="dsb", bufs=2) as dsb, \
             tc.tile_pool(name="drl", bufs=2) as drlp, \
             tc.tile_pool(name="dps", bufs=1, space="PSUM") as dps, \
             tc.tile_pool(name="dqps", bufs=2, space="PSUM") as dqps:
            Wq_s = ph4w.tile([128, HK, A + G], BF16)
            nc.gpsimd.dma_start(out=Wq_s[:, 0:HK // 2, :],
                in_=WqWhhT[0:H // 2, :].rearrange("(k p) g -> p k g", p=128))
            nc.scalar.dma_start(out=Wq_s[:, HK // 2:HK, :],
                in_=WqWhhT[H // 2:H, :].rearrange("(k p) g -> p k g", p=128))
            qsb = ph4w.tile([128, A], BF16)
            hrep = ph4w.tile([128, HK * 128], BF16)
            scoreP = ph4w.tile([128, NQ], F32)
            scoreN = ph4w.tile([128, NQ], F32)
            ssum = ph4w.tile([128, NQ], F32)
            mexp = ph4w.tile([128, NQ], F32)
            wTm = ph4w.tile([128, NQ, Bl], BF16)
            rec = ph4w.tile([Bl, 1], F32)
            hbf = ph4w.tile([Bl, H], BF16)
            nc.vector.tensor_copy(hbf[:, :], h[:, :])

            def dec_step(t_expr):
                xst = dxstp.tile([Bl, G], BF16, tag="dx")
                nc.sync.dma_start(
                    out=xst[:, :], in_=Xd_d[ds(t_expr, 1)].flatten_outer_dims())
                # replicated h transposes: scr[:, kc*128+(s',b)] = H[b, kc*128+p]
                scr = dps.tile([128, 512], F32, space="PSUM", tag="scr")
                aux = dps.tile([128, 512], F32, space="PSUM", tag="aux")
                for kc in range(HK):
                    nc.tensor.matmul(out=scr[:, 128 * kc:128 * (kc + 1)],
                                     lhsT=hbf[:, 128 * kc:128 * (kc + 1)],
                                     rhs=rep4_b[:, :], start=True, stop=True)
                nc.scalar.copy(hrep[:, :], scr[:, 0:512])
                # q proj (sb-major): out[sb, a]
                for n in range(AN):
                    qp = dqps.tile([128, 512], F32, space="PSUM", tag="qps")
                    for kc in range(HK):
                        nc.tensor.matmul(
                            out=qp[:, :],
                            lhsT=hrep[:, 128 * kc:128 * (kc + 1)],
                            rhs=Wq_s[:, kc, 512 * n:512 * (n + 1)],
                            start=(kc == 0), stop=(kc == HK - 1))
                    nc.scalar.copy(qsb[:, 512 * n:512 * (n + 1)], qp[:, :])
                # gates: h-part + x-part (ctx part accumulates later);
                # one PSUM tile per gate slice so reads unblock per-slice
                gt = [dps.tile([128, 512], F32, space="PSUM", tag="dg%d" % n,
                               name="gt%d" % n) for n in range(GN)]
                for n in range(GN):
                    gsl = gt[n][0:Bl, :]
                    for kc in range(HK):
                        nc.tensor.matmul(
                            out=gsl, lhsT=hrep[:, 128 * kc:128 * kc + Bl],
                            rhs=Wq_s[:, kc, A + 512 * n:A + 512 * (n + 1)],
                            start=(kc == 0), stop=False)
                    nc.tensor.matmul(out=gsl, lhsT=I4b[:, :],
                                     rhs=xst[:, 512 * n:512 * (n + 1)],
                                     start=False, stop=False)
                # attention: rl = relu(u + q); score = sum(pos) - sum(neg)
                for cq in range(NQ):
                    rl = drlp.tile([128, A], BF16, tag="rl%d" % cq)
                    if cq < 2:
                        # per-a-chunk adds so each starts as soon as its
                        # qsb chunk is copied
                        for n in range(AN):
                            nc.vector.tensor_tensor(
                                out=rl[:, 512 * n:512 * (n + 1)],
                                in0=u_sb[:, cq, 512 * n:512 * (n + 1)],
                                in1=qsb[:, 512 * n:512 * (n + 1)], op=ALU.add)
                        nc.scalar.activation(
                            out=rl[:, 0:npos], in_=rl[:, 0:npos], func=AF.Relu,
                            accum_out=scoreP[:, cq:cq + 1])
                        nc.scalar.activation(
                            out=rl[:, npos:A], in_=rl[:, npos:A], func=AF.Relu,
                            accum_out=scoreN[:, cq:cq + 1])
                    else:
                        nc.vector.tensor_tensor(out=rl[:, :], in0=u_sb[:, cq, :],
                                                in1=qsb[:, :], op=ALU.add)
                        nc.vector.tensor_scalar(
                            out=rl[:, 0:npos], in0=rl[:, 0:npos],
                            scalar1=0.0, scalar2=None, op0=ALU.max, op1=ALU.add,
                            accum_out=scoreP[:, cq:cq + 1])
                        nc.vector.tensor_scalar(
                            out=rl[:, npos:A], in0=rl[:, npos:A],
                            scalar1=0.0, scalar2=None, op0=ALU.max, op1=ALU.add,
                            accum_out=scoreN[:, cq:cq + 1])
                # per-tile: score diff -> exp -> denominator accumulate,
                # each as soon as that tile's accumulates land
                for cq in range(NQ):
                    nc.vector.scalar_tensor_tensor(
                        out=ssum[:, cq:cq + 1], in0=scoreN[:, cq:cq + 1],
                        scalar=-1.0, in1=scoreP[:, cq:cq + 1],
                        op0=ALU.mult, op1=ALU.add)
                    nc.scalar.activation(out=mexp[:, cq:cq + 1],
                                         in_=ssum[:, cq:cq + 1], func=AF.Exp)
                    nc.tensor.matmul(
                        out=aux[0:Bl, 8:9],
                        lhsT=(mask4v_s if cq == NQ - 1 else mask4_s)[:, :],
                        rhs=mexp[:, cq:cq + 1],
                        start=(cq == 0), stop=(cq == NQ - 1))
                nc.vector.reciprocal(rec[:, :], aux[0:Bl, 8:9])
                nc.tensor.matmul(out=aux[:, 16:17], lhsT=rep4_s[:, :],
                                 rhs=rec[:, :], start=True, stop=True)
                nc.vector.scalar_tensor_tensor(
                    out=wTm[:, :, :], in0=mask4F_s[:, :, :],
                    scalar=aux[:, 16:17], op0=ALU.mult,
                    in1=mexp[:, :].to_broadcast([128, NQ, Bl]), op1=ALU.mult)
                # ctx-part of gates (f,g,i,o: both cell product paths early)
                for n in (1, 2, 0, 3):
                    gsl = gt[n][0:Bl, :]
                    for cq in range(NQ):
                        nc.tensor.matmul(
                            out=gsl, lhsT=(wTm[:, cq, :]),
                            rhs=(encWcQ[:, cq, 512 * n:512 * (n + 1)]),
                            start=False, stop=(cq == NQ - 1))
                # cell update, 2x domain:
                #   C' = 0.5*(1+tf)*C + (1+ti)*tanh(g);  H' = (1+to)*tanh(C'/2)
                tf_ = dcellp.tile([Bl, H], F32, tag="tf")
                nc.scalar.activation(out=tf_[:, :], in_=gt[1][0:Bl, :],
                                     func=AF.Tanh, scale=0.5)
                tg = dcellp.tile([Bl, H], F32, tag="tg")
                nc.scalar.activation(out=tg[:, :], in_=gt[2][0:Bl, :],
                                     func=AF.Tanh)
                p2 = dcellp.tile([Bl, H], F32, tag="p2")
                nc.vector.scalar_tensor_tensor(
                    out=p2[:, :], in0=tf_[:, :], scalar=1.0, in1=c[:, :],
                    op0=ALU.add, op1=ALU.mult)
                ti_ = dcellp.tile([Bl, H], F32, tag="ti")
                nc.scalar.activation(out=ti_[:, :], in_=gt[0][0:Bl, :],
                                     func=AF.Tanh, scale=0.5)
                to_ = dcellp.tile([Bl, H], F32, tag="to")
                nc.scalar.activation(out=to_[:, :], in_=gt[3][0:Bl, :],
                                     func=AF.Tanh, scale=0.5)
                p1 = dcellp.tile([Bl, H], F32, tag="p1")
                nc.vector.scalar_tensor_tensor(
                    out=p1[:, :], in0=ti_[:, :], scalar=1.0, in1=tg[:, :],
                    op0=ALU.add, op1=ALU.mult)
                nc.vector.scalar_tensor_tensor(
                    out=c[:, :], in0=p2[:, :], scalar=0.5, in1=p1[:, :],
                    op0=ALU.mult, op1=ALU.add)
                tcn = dcellp.tile([Bl, H], F32, tag="tc")
                nc.scalar.activation(out=tcn[:, :], in_=c[:, :],
                                     func=AF.Tanh, scale=0.5)
                nc.vector.scalar_tensor_tensor(
                    out=hbf[:, :], in0=to_[:, :], scalar=1.0, in1=tcn[:, :],
                    op0=ALU.add, op1=ALU.mult)
                nc.sync.dma_start(
                    out=dec_d[ds(t_expr, 1)].flatten_outer_dims(), in_=hbf[:, :])
                # ctx output (for classifier); emitted late to keep the PE
                # busy (and warm) between pctx and next step's transposes
                ctst = dsb.tile([Bl, 2 * H], BF16, tag="ctst")
                for cq in range(NQ):
                    nc.tensor.matmul(out=aux[0:Bl, 0:512], lhsT=(wTm[:, cq, :]),
                                     rhs=(ofQ[:, cq, :]),
                                     start=(cq == 0), stop=(cq == NQ - 1))
                nc.scalar.copy(ctst[:, 0:512], aux[0:Bl, 0:512])
                for cq in range(NQ):
                    nc.tensor.matmul(out=aux[0:Bl, 0:512], lhsT=(wTm[:, cq, :]),
                                     rhs=(obQ[:, cq, :]),
                                     start=(cq == 0), stop=(cq == NQ - 1))
                nc.scalar.copy(ctst[:, 512:1024], aux[0:Bl, 0:512])
                nc.sync.dma_start(
                    out=ctx_d[ds(t_expr, 1)].flatten_outer_dims(), in_=ctst[:, :])

            with tc.For_i(0, T, unroll) as t0:
                for u_ in range(unroll):
                    dec_step(t0 + u_)

        scopeB_cm.__exit__(None, None, None)

        # ================= PHASE 5: classifier =================
        with tc.tile_pool(name="ph5", bufs=1) as ph5, \
             tc.tile_pool(name="ph5st", bufs=2) as ph5st, \
             tc.tile_pool(name="c1wp", bufs=1) as c1wp, \
             tc.tile_pool(name="ph5ps", bufs=1, space="PSUM") as ph5ps, \
             tc.tile_pool(name="c2psp", bufs=2, space="PSUM") as c2psp, \
             tc.tile_pool(name="ph5o", bufs=2) as ph5o:
            ctxQ = ph5.tile([128, NQT, 2 * H], BF16)
            decQ = ph5.tile([128, NQT, H], BF16)
            nc.vector.memset(ctxQ[:, :, :], 0.0)
            nc.vector.memset(decQ[:, :, :], 0.0)
            for q in range(NQT):
                rows = min(32, T - 32 * q) * Bl
                nc.sync.dma_start(
                    out=ctxQ[:rows, q, :],
                    in_=ctx_d[32 * q:32 * q + rows // Bl].flatten_outer_dims())
                nc.sync.dma_start(
                    out=decQ[:rows, q, :],
                    in_=dec_d[32 * q:32 * q + rows // Bl].flatten_outer_dims())
            ctxT = ph5.tile([128, H2K, 128 * NQT], BF16)
            decT = ph5.tile([128, HK, 128 * NQT], BF16)
            for q in range(NQT):
                pT = ph5ps.tile([128, H2K, 128], BF16, space="PSUM", tag="tps")
                for kc in range(H2K):
                    nc.tensor.transpose(out=pT[:, kc, :],
                                        in_=ctxQ[:, q, 128 * kc:128 * (kc + 1)],
                                        identity=I128b[:, :])
                nc.vector.tensor_copy(ctxT[:, :, 128 * q:128 * (q + 1)], pT[:, :, :])
                pT2 = ph5ps.tile([128, HK, 128], BF16, space="PSUM", tag="tps2")
                for kc in range(HK):
                    nc.tensor.transpose(out=pT2[:, kc, :],
                                        in_=decQ[:, q, 128 * kc:128 * (kc + 1)],
                                        identity=I128b[:, :])
                nc.vector.tensor_copy(decT[:, :, 128 * q:128 * (q + 1)], pT2[:, :, :])

            cls1b_s = ph5.tile([1, 1024], BF16)
            nc.gpsimd.dma_start(out=cls1b_s[:, :], in_=cls1b[:, :])
            h1Q = ph5.tile([128, NQT, 1024], BF16)
            lhs_chunks = ([(tembT, kc) for kc in range(EK)]
                          + [(ctxT, kc) for kc in range(H2K)]
                          + [(decT, kc) for kc in range(HK)])
            for n in range(2):
                c1w = c1wp.tile([128, G // 128, 512], BF16, tag="c1w")
                nc.gpsimd.dma_start(out=c1w[:, :, :],
                    in_=cls1WT[:, 512 * n:512 * (n + 1)].rearrange(
                        "(k p) n2 -> p k n2", p=128))
                for Tt in range(NQT):
                    ps = ph5ps.tile([128, 512], F32, space="PSUM", tag="c1ps")
                    for kg, (lt, kc) in enumerate(lhs_chunks):
                        nc.tensor.matmul(
                            out=ps[:, :],
                            lhsT=(lt[:, kc, 128 * Tt:128 * (Tt + 1)]),
                            rhs=(c1w[:, kg, :]),
                            start=(kg == 0), stop=False)
                    nc.tensor.matmul(out=ps[:, :], lhsT=(ones[:1, :128]),
                                     rhs=(cls1b_s[:, 512 * n:512 * (n + 1)]),
                                     start=False, stop=True)
                    nc.scalar.activation(out=h1Q[:, Tt, 512 * n:512 * (n + 1)],
                                         in_=ps[:, :], func=AF.Relu)
            h1T = ph5.tile([128, 8, 128 * NQT], BF16)
            for q in range(NQT):
                pT = ph5ps.tile([128, 8, 128], BF16, space="PSUM", tag="tps3")
                for kc in range(8):
                    nc.tensor.transpose(out=pT[:, kc, :],
                                        in_=h1Q[:, q, 128 * kc:128 * (kc + 1)],
                                        identity=I128b[:, :])
                nc.vector.tensor_copy(h1T[:, :, 128 * q:128 * (q + 1)], pT[:, :, :])

            def cls2_chunk(nv, nw):
                Wc = ph5st.tile([128, 8, 512], BF16, tag="c2w")
                # split the 1MB weight load across two DMA queues
                nc.gpsimd.dma_start(
                    out=Wc[:, 0:4, :nw],
                    in_=cls2WT[0:512, ds(nv, nw)].rearrange(
                        "(k p) n -> p k n", p=128))
                nc.scalar.dma_start(
                    out=Wc[:, 4:8, :nw],
                    in_=cls2WT[512:1024, ds(nv, nw)].rearrange(
                        "(k p) n -> p k n", p=128))
                bc_ = ph5st.tile([128, 512], BF16, tag="c2b")
                nc.gpsimd.dma_start(out=bc_[:, :nw], in_=cls2bR[:, ds(nv, nw)])
                for Tt in range(NQT):
                    trows = min(32, T - 32 * Tt)
                    ps = c2psp.tile([128, 512], F32, space="PSUM", tag="c2ps")
                    for kc in range(8):
                        nc.tensor.matmul(
                            out=ps[:, :nw],
                            lhsT=(h1T[:, kc, 128 * Tt:128 * (Tt + 1)]),
                            rhs=(Wc[:, kc, :nw]), start=(kc == 0), stop=(kc == 7))
                    ost = ph5o.tile([128, 512], BF16, tag="ost")
                    nc.vector.tensor_tensor(out=ost[:, :nw], in0=ps[:, :nw],
                                            in1=bc_[:, :nw], op=ALU.add)
                    nc.sync.dma_start(
                        out=logits[:, :, :].flatten_outer_dims()[
                            128 * Tt:128 * Tt + trows * Bl, ds(nv, nw)],
                        in_=ost[:trows * Bl, :nw])

            nfull = V // 512
            nd = (nfull // 4) * 4
            if nd > 0:
                with tc.For_i(0, 512 * nd, 2048) as nv0:
                    for uu in range(4):
                        cls2_chunk(nv0 + 512 * uu, 512)
            for start in range(512 * nd, V, 512):
                cls2_chunk(start, min(512, V - start))

        _stack.close()
    return nc


def prep_shared(p, V):
    """Core-independent input prep. p: dict of full-model params."""
    d = {}
    d["src_emb"] = np.ascontiguousarray(p["src_emb"], np.float32)
    d["tgt_emb"] = np.ascontiguousarray(p["tgt_emb"], np.float32)
    d["WihfT"] = np.ascontiguousarray(p["enc_Wih_f"].T).astype(BF)
    d["WihbT"] = np.ascontiguousarray(p["enc_Wih_b"].T).astype(BF)
    d["WhhfT"] = np.ascontiguousarray(p["enc_Whh_f"].T).astype(BF)
    d["WhhbT"] = np.ascontiguousarray(p["enc_Whh_b"].T).astype(BF)
    d["biasf"] = (p["enc_bih_f"] + p["enc_bhh_f"]).astype(BF)[None, :]
    d["biasb"] = (p["enc_bih_b"] + p["enc_bhh_b"]).astype(BF)[None, :]
    d["WiheT"] = np.ascontiguousarray(p["dec_Wih"][:, :E].T).astype(BF)
    d["WihcT"] = np.ascontiguousarray(p["dec_Wih"][:, E:].T).astype(BF)
    d["biasd"] = (p["dec_bih"] + p["dec_bhh"]).astype(BF)[None, :]
    # attention: fold |a2| into A1 rows (permuted pos-first); fold 0.5
    # (decoder 2x h-domain) into the h-contracting weights
    a2 = np.asarray(p["att2_W"][0], np.float32)
    perm = np.argsort(~(a2 > 0), kind="stable")     # positives first
    npos = int((a2 > 0).sum())
    sc = np.abs(a2[perm])[:, None]
    A1p = np.asarray(p["att1_W"], np.float32)[perm]
    d["A1eT"] = np.ascontiguousarray((A1p[:, :2 * H] * sc).T).astype(BF)
    d["att1b"] = (np.asarray(p["att1_b"], np.float32)[perm] * sc[:, 0]
                  ).astype(BF)[None, :]
    d["WqWhhT"] = np.ascontiguousarray(
        np.concatenate([(A1p[:, 2 * H:] * sc).T * 0.5,
                        p["dec_Whh"].T * 0.5], axis=1)).astype(BF)
    d["_npos"] = npos
    cls1 = np.asarray(p["cls1_W"], np.float32).copy()
    cls1[:, E + 2 * H:] *= 0.5          # decoder h stored as 2h
    d["cls1WT"] = np.ascontiguousarray(cls1.T).astype(BF)
    d["cls1b"] = p["cls1_b"].astype(BF)[None, :]
    d["cls2WT"] = np.ascontiguousarray(p["cls2_W"].T).astype(BF)
    d["cls2bR"] = np.broadcast_to(p["cls2_b"].astype(BF)[None, :],
                                  (128, V)).copy()
    mask4 = np.zeros((128, Bl), np.float32)
    for pp in range(128):
        mask4[pp, pp % Bl] = 1.0
    d["mask4"] = mask4
    S = 100
    NQ = (S + 31) // 32
    valid3 = (np.arange(128) // Bl + 32 * (NQ - 1)) < S
    d["mask4v"] = mask4 * valid3[:, None].astype(np.float32)
    m4F = np.concatenate([mask4] * (NQ - 1) + [d["mask4v"]], axis=1)
    d["mask4F"] = np.ascontiguousarray(m4F, np.float32)
    rep4 = np.zeros((Bl, 128), np.float32)
    for pp in range(128):
        rep4[pp % Bl, pp] = 1.0
    d["rep4"] = rep4
    return d


def idx_tile(tok, S):
    """tok: [Bl, S] int array -> [128, NQ] int32, token p=4*s'+b."""
    NQ = (S + 31) // 32
    out = np.zeros((128, NQ), np.int32)
    for q in range(NQ):
        for pp in range(128):
            b, sp = pp % Bl, pp // Bl
            s = 32 * q + sp
            if s < S:
                out[pp, q] = tok[b, s]
    return out


def vmask_tile(S):
    NQ = (S + 31) // 32
    out = np.zeros((128, NQ), np.float32)
    for q in range(NQ):
        for pp in range(128):
            if 32 * q + pp // Bl < S:
                out[pp, q] = 1.0
    return out


def prep_core(shared, source_data, target_data, core, S, T):
    d = {k: v for k, v in shared.items() if not k.startswith("_")}
    d["idx_src"] = idx_tile(source_data[4 * core:4 * core + 4], S)
    d["idx_tgt"] = idx_tile(target_data[4 * core:4 * core + 4], T)
    return d


def np_reference(src, tgt, p):
    """Port of reference.py for batch rows in src/tgt [B, S]."""
    def sig(x): return 1.0 / (1.0 + np.exp(-x))

    def lstm_step(x, h, c, Wih, Whh, bih, bhh):
        g = x @ Wih.T + h @ Whh.T + (bih + bhh)
        i, f, gg, o = np.split(g, 4, axis=-1)
        c = sig(f) * c + sig(i) * np.tanh(gg)
        h = sig(o) * np.tanh(c)
        return h, c

    B, S = src.shape
    T = tgt.shape[1]
    x = p["src_emb"][src].transpose(1, 0, 2).astype(np.float32)
    z = np.zeros((B, H), np.float32)
    hf, cf = z, z
    of = []
    for s in range(S):
        hf, cf = lstm_step(x[s], hf, cf, p["enc_Wih_f"], p["enc_Whh_f"],
                           p["enc_bih_f"], p["enc_bhh_f"])
        of.append(hf)
    hb, cb = z, z
    ob = []
    for s in range(S):
        hb, cb = lstm_step(x[S - 1 - s], hb, cb, p["enc_Wih_b"], p["enc_Whh_b"],
                           p["enc_bih_b"], p["enc_bhh_b"])
        ob.append(hb)
    of = np.stack(of); ob = np.stack(ob)
    enc = np.concatenate([of, ob[::-1]], -1).transpose(1, 0, 2)  # [B,S,2H]
    temb = p["tgt_emb"][tgt].astype(np.float32)                  # [B,T,E]
    h, c = hf, cf
    ctxs, decs = [], []
    for t in range(T):
        prev = np.broadcast_to(h[:, None, :], (B, S, H))
        ain = np.concatenate([enc, prev], -1)
        hid = np.maximum(ain @ p["att1_W"].T + p["att1_b"], 0.0)
        sc = hid @ p["att2_W"].T + p["att2_b"]
        w = np.exp(sc - sc.max(axis=1, keepdims=True))
        w = w / w.sum(axis=1, keepdims=True)
        ctx = (w * enc).sum(axis=1)
        h, c = lstm_step(np.concatenate([temb[:, t], ctx], -1), h, c,
                         p["dec_Wih"], p["dec_Whh"], p["dec_bih"], p["dec_bhh"])
        ctxs.append(ctx); decs.append(h)
    ctxs = np.stack(ctxs, 1); decs = np.stack(decs, 1)
    ci = np.concatenate([temb, ctxs, decs], -1)
    h1 = np.maximum(ci @ p["cls1_W"].T + p["cls1_b"], 0.0)
    return h1 @ p["cls2_W"].T + p["cls2_b"]

# ===================== host-side entry point =====================
_CACHE = {}


def _get_nc(npos):
    if "nc" not in _CACHE:
        nc = build_nc(S=100, T=100, V=32000, num_devices=8, unroll=10,
                      npos=npos)
        nc.compile()
        _CACHE["nc"] = nc
    return _CACHE["nc"]


def kernel(trace=False, **inputs):
    S = T = 100
    V = 32000
    B = 32
    from concourse.bass_utils import run_bass_kernel_spmd
    shared = prep_shared(inputs, V)
    nc = _get_nc(shared["_npos"])
    src = np.asarray(inputs["source_data"])
    tgt = np.asarray(inputs["target_data"])
    in_maps = [prep_core(shared, src, tgt, c, S, T) for c in range(8)]
    res = run_bass_kernel_spmd(nc, in_maps, core_ids=list(range(8)),
                               trace=trace)
    out = np.empty((B, T, V), np.float32)
    for c in range(8):
        lg = np.asarray(res.results[c]["logits"]).astype(
            np.float32).reshape(T, Bl, V)
        out[4 * c:4 * c + 4] = lg.transpose(1, 0, 2)
    if trace:
        _CACHE["exec_time_ns"] = res.exec_time_ns
        _CACHE["profile"] = res
    return out


# revision 37
# speedup vs baseline: 1.1723x; 1.1723x over previous
# Self-contained TRN2 Bass kernel for nn_Attention_NMT (B=32,S=T=100,H=E=512,V=32000).
# SPMD over 8 NeuronCores, batch-parallel (4 batch rows per core).
# v2: sb-major attention (scores via DVE accumulate), tanh-domain LSTM cell
# (no act-table swaps), bias folded into cls2 copies.
import sys
for _p in ("/opt/trn_rl_repo",):
    if _p not in sys.path:
        sys.path.insert(0, _p)
import numpy as np
import ml_dtypes
BF = ml_dtypes.bfloat16
import concourse.bass as bass
import concourse.bacc as bacc
import concourse.tile as tile
from concourse import mybir
from concourse.bass import ds
from concourse.masks import make_identity

F32 = mybir.dt.float32
BF16 = mybir.dt.bfloat16
I32 = mybir.dt.int32
AF = mybir.ActivationFunctionType
ALU = mybir.AluOpType

E = 512; H = 512; G = 2048; A = 1536; Bl = 4
EK = E // 128; HK = H // 128; H2K = 2 * H // 128; AJ = A // 128
GN = G // 512; AN = A // 512


def build_nc(S=100, T=100, V=32000, num_devices=8, unroll=4, npos=768):
    NQ = (S + 31) // 32          # source quarters / token tiles
    NQT = (T + 31) // 32
    SP = 32 * NQ                 # padded
    TP = 32 * NQT

    nc = bacc.Bacc("TRN2", target_bir_lowering=False, debug=False,
                   num_devices=num_devices)

    def din(name, shape, dt=F32):
        return nc.dram_tensor(name, shape, dt, kind="ExternalInput")

    src_emb = din("src_emb", [V, E]); tgt_emb = din("tgt_emb", [V, E])
    idx_src = din("idx_src", [128, NQ], I32)
    idx_tgt = din("idx_tgt", [128, NQT], I32)
    WihfT = din("WihfT", [E, G], BF16); WihbT = din("WihbT", [E, G], BF16)
    WiheT = din("WiheT", [E, G], BF16)
    WhhfT = din("WhhfT", [H, G], BF16); WhhbT = din("WhhbT", [H, G], BF16)
    WqWhhT = din("WqWhhT", [H, A + G], BF16)
    WihcT = din("WihcT", [2 * H, G], BF16)
    A1eT = din("A1eT", [2 * H, A], BF16)
    biasf = din("biasf", [1, G], BF16); biasb = din("biasb", [1, G], BF16)
    biasd = din("biasd", [1, G], BF16); att1b = din("att1b", [1, A], BF16)
    cls1WT = din("cls1WT", [G, 1024], BF16); cls1b = din("cls1b", [1, 1024], BF16)
    cls2WT = din("cls2WT", [1024, V], BF16)
    cls2bR = din("cls2bR", [128, V], BF16)
    mask4 = din("mask4", [128, Bl])
    mask4v = din("mask4v", [128, Bl])
    mask4F = din("mask4F", [128, (S + 31) // 32 * Bl])
    rep4 = din("rep4", [Bl, 128])

    logits = nc.dram_tensor("logits", [T, Bl, V], BF16, kind="ExternalOutput")

    # DRAM scratch
    Xf_d = nc.dram_tensor("Xf_d", [S, Bl, G], BF16)
    Xb_d = nc.dram_tensor("Xb_d", [S, Bl, G], BF16)
    Xd_d = nc.dram_tensor("Xd_d", [T, Bl, G], BF16)
    of_d = nc.dram_tensor("of_d", [S, Bl, H], BF16)
    ob_d = nc.dram_tensor("ob_d", [S, Bl, H], BF16)
    ctx_d = nc.dram_tensor("ctx_d", [T, Bl, 2 * H], BF16)
    dec_d = nc.dram_tensor("dec_d", [T, Bl, H], BF16)

    with tile.TileContext(nc) as tc:
        from contextlib import ExitStack
        _stack = ExitStack()
        persist = _stack.enter_context(tc.tile_pool(name="persist", bufs=1))

        # ---- constants ----
        I128 = persist.tile([128, 128], F32)
        make_identity(nc, I128[:, :])
        ones = persist.tile([1, 512], BF16)
        nc.vector.memset(ones[:, :], 1.0)
        I128b = persist.tile([128, 128], BF16)
        nc.vector.tensor_copy(I128b[:, :], I128[:, :])
        mask4_s = persist.tile([128, Bl], F32)
        nc.gpsimd.dma_start(out=mask4_s[:, :], in_=mask4[:, :])
        rep4_s = persist.tile([Bl, 128], F32)
        nc.gpsimd.dma_start(out=rep4_s[:, :], in_=rep4[:, :])
        rep4_b = persist.tile([Bl, 128], BF16)
        nc.vector.tensor_copy(rep4_b[:, :], rep4_s[:, :])
        mask4v_s = persist.tile([128, Bl], F32)
        nc.gpsimd.dma_start(out=mask4v_s[:, :], in_=mask4v[:, :])
        mask4F_s = persist.tile([128, NQ, Bl], F32)
        nc.gpsimd.dma_start(out=mask4F_s[:, :, :],
                            in_=mask4F[:, :].rearrange("p (q b) -> p q b", b=Bl))
        I4b = persist.tile([Bl, Bl], BF16)
        nc.vector.tensor_copy(I4b[:, :], I128[:Bl, :Bl])
        att1b_s = persist.tile([1, A], BF16)
        nc.gpsimd.dma_start(out=att1b_s[:, :], in_=att1b[:, :])

        # persistent activations
        tembT = persist.tile([128, EK, 128 * NQT], BF16)
        h_f = persist.tile([Bl, H], F32); c_f = persist.tile([Bl, H], F32)
        h_b = persist.tile([Bl, H], F32); c_b = persist.tile([Bl, H], F32)
        for t_ in (h_f, c_f, h_b, c_b):
            nc.vector.memset(t_[:, :], 0.0)

        # ================= PHASE 0: embeddings + X GEMMs =================
        with tc.tile_pool(name="ph0", bufs=1) as ph0, \
             tc.tile_pool(name="ph0ps", bufs=2, space="PSUM") as ph0ps, \
             tc.tile_pool(name="ph0st", bufs=2) as ph0st:
            idxs = ph0.tile([128, NQ], I32)
            nc.gpsimd.dma_start(out=idxs[:, :], in_=idx_src[:, :])
            idxt = ph0.tile([128, NQT], I32)
            nc.gpsimd.dma_start(out=idxt[:, :], in_=idx_tgt[:, :])
            xQ = ph0.tile([128, NQ, E], F32)
            tembQ = ph0.tile([128, NQT, E], F32)
            for q in range(NQ):
                nc.gpsimd.indirect_dma_start(
                    out=xQ[:, q, :], out_offset=None, in_=src_emb[:, :],
                    in_offset=bass.IndirectOffsetOnAxis(ap=idxs[:, q:q + 1], axis=0))
            for q in range(NQT):
                nc.gpsimd.indirect_dma_start(
                    out=tembQ[:, q, :], out_offset=None, in_=tgt_emb[:, :],
                    in_offset=bass.IndirectOffsetOnAxis(ap=idxt[:, q:q + 1], axis=0))

            # transpose xQ/tembQ -> xT/tembT  (feature-major, token cols)
            xT = ph0.tile([128, EK, 128 * NQ], BF16)
            for q in range(NQ):
                pT = ph0ps.tile([128, EK, 128], F32, space="PSUM")
                for kc in range(EK):
                    nc.tensor.transpose(out=pT[:, kc, :],
                                        in_=xQ[:, q, 128 * kc:128 * (kc + 1)],
                                        identity=I128[:, :])
                nc.vector.tensor_copy(xT[:, :, 128 * q:128 * (q + 1)], pT[:, :, :])
            for q in range(NQT):
                pT = ph0ps.tile([128, EK, 128], F32, space="PSUM")
                for kc in range(EK):
                    nc.tensor.transpose(out=pT[:, kc, :],
                                        in_=tembQ[:, q, 128 * kc:128 * (kc + 1)],
                                        identity=I128[:, :])
                nc.vector.tensor_copy(tembT[:, :, 128 * q:128 * (q + 1)], pT[:, :, :])

            # X GEMMs -> DRAM   (token-stationary, stream W)
            def x_gemm(wT_dram, bias_dram, lhsT_tile, nQ, S_, out_dram):
                Ws = ph0.tile([128, EK, G], BF16, tag="ws_" + wT_dram.name)
                nc.gpsimd.dma_start(
                    out=Ws[:, 0:EK // 2, :],
                    in_=wT_dram[0:E // 2, :].rearrange("(k p) g -> p k g",
                                                       p=128))
                nc.scalar.dma_start(
                    out=Ws[:, EK // 2:EK, :],
                    in_=wT_dram[E // 2:E, :].rearrange("(k p) g -> p k g",
                                                       p=128))
                bia = ph0.tile([1, G], BF16, tag="bia_" + wT_dram.name)
                nc.gpsimd.dma_start(out=bia[:, :], in_=bias_dram[:, :])
                for Tt in range(nQ):
                    rows = min(32, S_ - 32 * Tt) * Bl
                    stage = ph0st.tile([128, G], BF16, tag="xstage")
                    for n in range(GN):
                        ps = ph0ps.tile([128, 512], F32, space="PSUM", tag="xps")
                        for kc in range(EK):
                            nc.tensor.matmul(
                                out=ps[:, :],
                                lhsT=(lhsT_tile[:, kc, 128 * Tt:128 * (Tt + 1)]),
                                rhs=(Ws[:, kc, 512 * n:512 * (n + 1)]),
                                start=(kc == 0), stop=False)
                        nc.tensor.matmul(
                            out=ps[:, :], lhsT=(ones[:1, :128]),
                            rhs=(bia[:, 512 * n:512 * (n + 1)]),
                            start=False, stop=True)
                        nc.scalar.copy(stage[:, 512 * n:512 * (n + 1)], ps[:, :])
                    nc.sync.dma_start(
                        out=out_dram[32 * Tt:32 * Tt + rows // Bl, :,
                                     :].flatten_outer_dims(),
                        in_=stage[:rows, :])
            x_gemm(WihfT, biasf, xT, NQ, S, Xf_d)
            x_gemm(WihbT, biasb, xT, NQ, S, Xb_d)
            x_gemm(WiheT, biasd, tembT, NQT, T, Xd_d)

        # ================= PHASE 1: encoder =================
        with tc.tile_pool(name="ph1w", bufs=1) as ph1w, \
             tc.tile_pool(name="xst", bufs=2) as xstp, \
             tc.tile_pool(name="cell", bufs=1) as cellp, \
             tc.tile_pool(name="hT", bufs=2) as hTp, \
             tc.tile_pool(name="encg", bufs=1, space="PSUM") as encg:
            Whhf_s = ph1w.tile([128, HK, G], BF16)
            nc.gpsimd.dma_start(out=Whhf_s[:, 0:HK // 2, :],
                in_=WhhfT[0:H // 2, :].rearrange("(k p) g -> p k g", p=128))
            nc.scalar.dma_start(out=Whhf_s[:, HK // 2:HK, :],
                in_=WhhfT[H // 2:H, :].rearrange("(k p) g -> p k g", p=128))
            Whhb_s = ph1w.tile([128, HK, G], BF16)
            nc.gpsimd.dma_start(out=Whhb_s[:, 0:HK // 2, :],
                in_=WhhbT[0:H // 2, :].rearrange("(k p) g -> p k g", p=128))
            nc.scalar.dma_start(out=Whhb_s[:, HK // 2:HK, :],
                in_=WhhbT[H // 2:H, :].rearrange("(k p) g -> p k g", p=128))

            def lstm_part1(h, c, Whh_s, X_d, s_expr, tagp):
                xst = xstp.tile([Bl, G], BF16, tag="xst" + tagp)
                nc.sync.dma_start(
                    out=xst[:, :], in_=X_d[ds(s_expr, 1)].flatten_outer_dims())
                gfull = encg.tile([128, G], F32, space="PSUM", tag="g" + tagp)
                for kc in range(HK):
                    nc.tensor.transpose(out=gfull[:, Bl * kc:Bl * (kc + 1)],
                                        in_=h[:, 128 * kc:128 * (kc + 1)],
                                        identity=I128[:Bl, :Bl])
                hTs = hTp.tile([128, HK, Bl], BF16, tag="hT" + tagp)
                nc.vector.tensor_copy(
                    hTs[:, :, :],
                    gfull[:, 0:HK * Bl].rearrange("p (k b) -> p k b", b=Bl))
                gates = gfull[0:Bl, :]
                # f-slice first so the cell chain starts early, then i, g, o
                for n in (1, 0, 2, 3):
                    gsl = gates[:, 512 * n:512 * (n + 1)]
                    for kc in range(HK):
                        nc.tensor.matmul(out=gsl, lhsT=hTs[:, kc, :],
                                         rhs=Whh_s[:, kc, 512 * n:512 * (n + 1)],
                                         start=(kc == 0), stop=False)
                    nc.tensor.matmul(out=gsl, lhsT=I4b[:, :],
                                     rhs=xst[:, 512 * n:512 * (n + 1)],
                                     start=False, stop=True)
                sf = cellp.tile([Bl, H], F32, tag="sf" + tagp)
                nc.scalar.activation(out=sf[:, :], in_=gates[:, 512:1024],
                                     func=AF.Sigmoid)
                si = cellp.tile([Bl, H], F32, tag="si" + tagp)
                nc.scalar.activation(out=si[:, :], in_=gates[:, 0:512],
                                     func=AF.Sigmoid)
                tg = cellp.tile([Bl, H], F32, tag="tg" + tagp)
                nc.scalar.activation(out=tg[:, :], in_=gates[:, 1024:1536],
                                     func=AF.Tanh)
                so = cellp.tile([Bl, H], F32, tag="so" + tagp)
                nc.scalar.activation(out=so[:, :], in_=gates[:, 1536:2048],
                                     func=AF.Sigmoid)
                p1 = cellp.tile([Bl, H], F32, tag="p1" + tagp)
                nc.vector.tensor_mul(p1[:, :], si[:, :], tg[:, :])
                p2 = cellp.tile([Bl, H], F32, tag="p2" + tagp)
                nc.vector.tensor_mul(p2[:, :], sf[:, :], c[:, :])
                nc.vector.tensor_add(c[:, :], p1[:, :], p2[:, :])
                return so

            def lstm_part2(h, c, so, s_expr, store_d, tagp):
                tcn = cellp.tile([Bl, H], F32, tag="tc" + tagp)
                nc.scalar.activation(out=tcn[:, :], in_=c[:, :], func=AF.Tanh)
                nc.vector.tensor_mul(h[:, :], so[:, :], tcn[:, :])
                if store_d is not None:
                    hbf = cellp.tile([Bl, H], BF16, tag="hbf" + tagp)
                    nc.vector.tensor_copy(hbf[:, :], h[:, :])
                    nc.sync.dma_start(
                        out=store_d[ds(s_expr, 1)].flatten_outer_dims(),
                        in_=hbf[:, :])

            with tc.For_i(0, S, 10) as i0:
                for u_ in range(10):
                    so_f = lstm_part1(h_f, c_f, Whhf_s, Xf_d, i0 + u_, "f")
                    so_b = lstm_part1(h_b, c_b, Whhb_s, Xb_d,
                                      (S - 1 - u_) - i0, "b")
                    lstm_part2(h_f, c_f, so_f, i0 + u_, of_d, "f")
                    lstm_part2(h_b, c_b, so_b, (S - 1 - u_) - i0, ob_d, "b")

        # decoder runs in the 2x domain: H = 2h, C = 2c
        nc.vector.tensor_scalar_mul(h_f[:, :], h_f[:, :], 2.0)
        nc.vector.tensor_scalar_mul(c_f[:, :], c_f[:, :], 2.0)

        # ============ PHASE 2: assemble enc tiles + transposes ============
        scopeB_cm = tc.tile_pool(name="scopeB", bufs=1)
        scopeB = scopeB_cm.__enter__()
        ofQ = scopeB.tile([128, NQ, H], BF16)
        obQ = scopeB.tile([128, NQ, H], BF16)
        u_sb = scopeB.tile([128, NQ, A], BF16)
        encWcQ = scopeB.tile([128, NQ, G], BF16)
        ph23_cm = tc.tile_pool(name="ph23", bufs=1)
        ph23 = ph23_cm.__enter__()
        encT = ph23.tile([128, H2K, 128 * NQ], BF16)
        with tc.tile_pool(name="ph2ps", bufs=2, space="PSUM") as ph2ps:
            nc.vector.memset(ofQ[:, :, :], 0.0)
            nc.vector.memset(obQ[:, :, :], 0.0)
            for q in range(NQ):
                rows = min(32, S - 32 * q) * Bl
                nc.sync.dma_start(
                    out=ofQ[:rows, q, :],
                    in_=of_d[32 * q:32 * q + rows // Bl].flatten_outer_dims())
                nc.sync.dma_start(
                    out=obQ[:rows, q, :],
                    in_=ob_d[32 * q:32 * q + rows // Bl].flatten_outer_dims())
            for q in range(NQ):
                pT = ph2ps.tile([128, HK, 128], BF16, space="PSUM")
                for kc in range(HK):
                    nc.tensor.transpose(out=pT[:, kc, :],
                                        in_=ofQ[:, q, 128 * kc:128 * (kc + 1)],
                                        identity=I128b[:, :])
                nc.vector.tensor_copy(encT[:, 0:HK, 128 * q:128 * (q + 1)],
                                      pT[:, :, :])
                pT2 = ph2ps.tile([128, HK, 128], BF16, space="PSUM")
                for kc in range(HK):
                    nc.tensor.transpose(out=pT2[:, kc, :],
                                        in_=obQ[:, q, 128 * kc:128 * (kc + 1)],
                                        identity=I128b[:, :])
                nc.vector.tensor_copy(encT[:, HK:H2K, 128 * q:128 * (q + 1)],
                                      pT2[:, :, :])

        # ============ PHASE 3: u GEMM (sb-major) + encWcQ GEMM ============
        with tc.tile_pool(name="ph3", bufs=1) as ph3, \
             tc.tile_pool(name="ph3st", bufs=3) as ph3st, \
             tc.tile_pool(name="ph3ps", bufs=2, space="PSUM") as ph3ps:
            A1e_s = ph3.tile([128, H2K, A], BF16)
            nc.gpsimd.dma_start(out=A1e_s[:, 0:H2K // 2, :],
                in_=A1eT[0:H, :].rearrange("(k p) a -> p k a", p=128))
            nc.scalar.dma_start(out=A1e_s[:, H2K // 2:H2K, :],
                in_=A1eT[H:2 * H, :].rearrange("(k p) a -> p k a", p=128))
            # u[sb, a] = enc[sb, :] @ A1e + b   (sb-major output)
            for cq in range(NQ):
                for n in range(AN):
                    ps = ph3ps.tile([128, 512], F32, space="PSUM", tag="ups")
                    for kc in range(H2K):
                        nc.tensor.matmul(
                            out=ps[:, :],
                            lhsT=(encT[:, kc, 128 * cq:128 * (cq + 1)]),
                            rhs=(A1e_s[:, kc, 512 * n:512 * (n + 1)]),
                            start=(kc == 0), stop=False)
                    nc.tensor.matmul(
                        out=ps[:, :], lhsT=(ones[:1, :128]),
                        rhs=(att1b_s[:, 512 * n:512 * (n + 1)]),
                        start=False, stop=True)
                    nc.scalar.copy(u_sb[:, cq, 512 * n:512 * (n + 1)], ps[:, :])
            # encWcQ: token-stationary, stream WihcT chunks from DRAM
            for n in range(GN):
                Wc_n = ph3st.tile([128, H2K, 512], BF16, tag="wcn")
                nc.gpsimd.dma_start(out=Wc_n[:, 0:H2K // 2, :],
                    in_=WihcT[0:H, 512 * n:512 * (n + 1)].rearrange(
                        "(k p) g -> p k g", p=128))
                nc.scalar.dma_start(out=Wc_n[:, H2K // 2:H2K, :],
                    in_=WihcT[H:2 * H, 512 * n:512 * (n + 1)].rearrange(
                        "(k p) g -> p k g", p=128))
                for Tt in range(NQ):
                    ps = ph3ps.tile([128, 512], F32, space="PSUM", tag="wcps")
                    for kc in range(H2K):
                        nc.tensor.matmul(
                            out=ps[:, :],
                            lhsT=(encT[:, kc, 128 * Tt:128 * (Tt + 1)]),
                            rhs=(Wc_n[:, kc, :]), start=(kc == 0),
                            stop=(kc == H2K - 1))
                    nc.scalar.copy(encWcQ[:, Tt, 512 * n:512 * (n + 1)], ps[:, :])

        ph23_cm.__exit__(None, None, None)

        # ================= PHASE 4: decoder =================
        h = h_f; c = c_f   # decoder init = final fwd encoder state (2x domain)
        nneg = A - npos
        with tc.tile_pool(name="ph4w", bufs=1) as ph4w, \
             tc.tile_pool(name="dxst", bufs=2) as dxstp, \
             tc.tile_pool(name="dcell", bufs=1) as dcellp, \
             tc.tile_pool(name="dsb", bufs=2) as dsb, \
             tc.tile_pool(name="drl", bufs=2) as drlp, \
             tc.tile_pool(name="dps", bufs=1, space="PSUM") as dps, \
             tc.tile_pool(name="dqps", bufs=2, space="PSUM") as dqps:
            Wq_s = ph4w.tile([128, HK, A + G], BF16)
            nc.gpsimd.dma_start(out=Wq_s[:, 0:HK // 2, :],
                in_=WqWhhT[0:H // 2, :].rearrange("(k p) g -> p k g", p=128))
            nc.scalar.dma_start(out=Wq_s[:, HK // 2:HK, :],
                in_=WqWhhT[H // 2:H, :].rearrange("(k p) g -> p k g", p=128))
            qsb = ph4w.tile([128, A], BF16)
            hrep = ph4w.tile([128, HK * 128], BF16)
            scoreP = ph4w.tile([128, NQ], F32)
            scoreN = ph4w.tile([128, NQ], F32)
            ssum = ph4w.tile([128, NQ], F32)
            mexp = ph4w.tile([128, NQ], F32)
            wTm = ph4w.tile([128, NQ, Bl], BF16)
            rec = ph4w.tile([Bl, 1], F32)
            hbf = ph4w.tile([Bl, H], BF16)
            nc.vector.tensor_copy(hbf[:, :], h[:, :])

            def dec_step(t_expr):
                xst = dxstp.tile([Bl, G], BF16, tag="dx")
                nc.sync.dma_start(
                    out=xst[:, :], in_=Xd_d[ds(t_expr, 1)].flatten_outer_dims())
                # replicated h transposes: scr[:, kc*128+(s',b)] = H[b, kc*128+p]
                scr = dps.tile([128, 512], F32, space="PSUM", tag="scr")
                aux = dps.tile([128, 512], F32, space="PSUM", tag="aux")
                for kc in range(HK):
                    nc.tensor.matmul(out=scr[:, 128 * kc:128 * (kc + 1)],
                                     lhsT=hbf[:, 128 * kc:128 * (kc + 1)],
                                     rhs=rep4_b[:, :], start=True, stop=True)
                nc.scalar.copy(hrep[:, :], scr[:, 0:512])
                # q proj (sb-major): out[sb, a]
                for n in range(AN):
                    qp = dqps.tile([128, 512], F32, space="PSUM", tag="qps")
                    for kc in range(HK):
                        nc.tensor.matmul(
                            out=qp[:, :],
                            lhsT=hrep[:, 128 * kc:128 * (kc + 1)],
                            rhs=Wq_s[:, kc, 512 * n:512 * (n + 1)],
                            start=(kc == 0), stop=(kc == HK - 1))
                    nc.scalar.copy(qsb[:, 512 * n:512 * (n + 1)], qp[:, :])
                # gates: h-part + x-part (ctx part accumulates later);
                # one PSUM tile per gate slice so reads unblock per-slice
                gt = [dps.tile([128, 512], F32, space="PSUM", tag="dg%d" % n,
                               name="gt%d" % n) for n in range(GN)]
                for n in range(GN):
                    gsl = gt[n][0:Bl, :]
                    for kc in range(HK):
                        nc.tensor.matmul(
                            out=gsl, lhsT=hrep[:, 128 * kc:128 * kc + Bl],
                            rhs=Wq_s[:, kc, A + 512 * n:A + 512 * (n + 1)],
                            start=(kc == 0), stop=False)
                    nc.tensor.matmul(out=gsl, lhsT=I4b[:, :],
                                     rhs=xst[:, 512 * n:512 * (n + 1)],
                                     start=False, stop=False)
                # attention: rl = relu(u + q); score = sum(pos) - sum(neg)
                for cq in range(NQ):
                    rl = drlp.tile([128, A], BF16, tag="rl%d" % cq)
                    if cq < 2:
                        # per-a-chunk adds so each starts as soon as its
                        # qsb chunk is copied
                        for n in range(AN):
                            nc.vector.tensor_tensor(
                                out=rl[:, 512 * n:512 * (n + 1)],
                                in0=u_sb[:, cq, 512 * n:512 * (n + 1)],
                                in1=qsb[:, 512 * n:512 * (n + 1)], op=ALU.add)
                        nc.scalar.activation(
                            out=rl[:, 0:npos], in_=rl[:, 0:npos], func=AF.Relu,
                            accum_out=scoreP[:, cq:cq + 1])
                        nc.scalar.activation(
                            out=rl[:, npos:A], in_=rl[:, npos:A], func=AF.Relu,
                            accum_out=scoreN[:, cq:cq + 1])
                    else:
                        nc.vector.tensor_tensor(out=rl[:, :], in0=u_sb[:, cq, :],
                                                in1=qsb[:, :], op=ALU.add)
                        nc.vector.tensor_scalar(
                            out=rl[:, 0:npos], in0=rl[:, 0:npos],
                            scalar1=0.0, scalar2=None, op0=ALU.max, op1=ALU.add,
                            accum_out=scoreP[:, cq:cq + 1])
                        nc.vector.tensor_scalar(
                            out=rl[:, npos:A], in0=rl[:, npos:A],
                            scalar1=0.0, scalar2=None, op0=ALU.max, op1=ALU.add,
                            accum_out=scoreN[:, cq:cq + 1])
                # per-tile: score diff -> exp -> denominator accumulate,
                # each as soon as that tile's accumulates land
                for cq in range(NQ):
                    nc.vector.scalar_tensor_tensor(
                        out=ssum[:, cq:cq + 1], in0=scoreN[:, cq:cq + 1],
                        scalar=-1.0, in1=scoreP[:, cq:cq + 1],
                        op0=ALU.mult, op1=ALU.add)
                    nc.scalar.activation(out=mexp[:, cq:cq + 1],
                                         in_=ssum[:, cq:cq + 1], func=AF.Exp)
                    nc.tensor.matmul(
                        out=aux[0:Bl, 8:9],
                        lhsT=(mask4v_s if cq == NQ - 1 else mask4_s)[:, :],
                        rhs=mexp[:, cq:cq + 1],
                        start=(cq == 0), stop=(cq == NQ - 1))
                nc.vector.reciprocal(rec[:, :], aux[0:Bl, 8:9])
                nc.tensor.matmul(out=aux[:, 16:17], lhsT=rep4_s[:, :],
                                 rhs=rec[:, :], start=True, stop=True)
                nc.vector.scalar_tensor_tensor(
                    out=wTm[:, :, :], in0=mask4F_s[:, :, :],
                    scalar=aux[:, 16:17], op0=ALU.mult,
                    in1=mexp[:, :].to_broadcast([128, NQ, Bl]), op1=ALU.mult)
                # ctx-part of gates (f,g,i,o: both cell product paths early)
                for n in (1, 2, 0, 3):
                    gsl = gt[n][0:Bl, :]
                    for cq in range(NQ):
                        nc.tensor.matmul(
                            out=gsl, lhsT=(wTm[:, cq, :]),
                            rhs=(encWcQ[:, cq, 512 * n:512 * (n + 1)]),
                            start=False, stop=(cq == NQ - 1))
                # cell update, 2x domain:
                #   C' = 0.5*(1+tf)*C + (1+ti)*tanh(g);  H' = (1+to)*tanh(C'/2)
                tf_ = dcellp.tile([Bl, H], F32, tag="tf")
                nc.scalar.activation(out=tf_[:, :], in_=gt[1][0:Bl, :],
                                     func=AF.Tanh, scale=0.5)
                tg = dcellp.tile([Bl, H], F32, tag="tg")
                nc.scalar.activation(out=tg[:, :], in_=gt[2][0:Bl, :],
                                     func=AF.Tanh)
                p2 = dcellp.tile([Bl, H], F32, tag="p2")
                nc.vector.scalar_tensor_tensor(
                    out=p2[:, :], in0=tf_[:, :], scalar=1.0, in1=c[:, :],
                    op0=ALU.add, op1=ALU.mult)
                ti_ = dcellp.tile([Bl, H], F32, tag="ti")
                nc.scalar.activation(out=ti_[:, :], in_=gt[0][0:Bl, :],
                                     func=AF.Tanh, scale=0.5)
                to_ = dcellp.tile([Bl, H], F32, tag="to")
                nc.scalar.activation(out=to_[:, :], in_=gt[3][0:Bl, :],
                                     func=AF.Tanh, scale=0.5)
                p1 = dcellp.tile([Bl, H], F32, tag="p1")
                nc.vector.scalar_tensor_tensor(
                    out=p1[:, :], in0=ti_[:, :], scalar=1.0, in1=tg[:, :],
                    op0=ALU.add, op1=ALU.mult)
                nc.vector.scalar_tensor_tensor(
                    out=c[:, :], in0=p2[:, :], scalar=0.5, in1=p1[:, :],
                    op0=ALU.mult, op1=ALU.add)
                tcn = dcellp.tile([Bl, H], F32, tag="tc")
                nc.scalar.activation(out=tcn[:, :], in_=c[:, :],
                                     func=AF.Tanh, scale=0.5)
                nc.vector.scalar_tensor_tensor(
                    out=hbf[:, :], in0=to_[:, :], scalar=1.0, in1=tcn[:, :],
                    op0=ALU.add, op1=ALU.mult)
                nc.sync.dma_start(
                    out=dec_d[ds(t_expr, 1)].flatten_outer_dims(), in_=hbf[:, :])
                # ctx output (for classifier); emitted late to keep the PE
                # busy (and warm) between pctx and next step's transposes
                ctst = dsb.tile([Bl, 2 * H], BF16, tag="ctst")
                for cq in range(NQ):
                    nc.tensor.matmul(out=aux[0:Bl, 0:512], lhsT=(wTm[:, cq, :]),
                                     rhs=(ofQ[:, cq, :]),
                                     start=(cq == 0), stop=(cq == NQ - 1))
                nc.scalar.copy(ctst[:, 0:512], aux[0:Bl, 0:512])
                for cq in range(NQ):
                    nc.tensor.matmul(out=aux[0:Bl, 0:512], lhsT=(wTm[:, cq, :]),
                                     rhs=(obQ[:, cq, :]),
                                     start=(cq == 0), stop=(cq == NQ - 1))
                nc.scalar.copy(ctst[:, 512:1024], aux[0:Bl, 0:512])
                nc.sync.dma_start(
                    out=ctx_d[ds(t_expr, 1)].flatten_outer_dims(), in_=ctst[:, :])

            with tc.For_i(0, T, unroll) as t0:
                for u_ in range(unroll):
                    dec_step(t0 + u_)

        scopeB_cm.__exit__(None, None, None)

        # ================= PHASE 5: classifier =================
        with tc.tile_pool(name="ph5", bufs=1) as ph5, \
             tc.tile_pool(name="ph5st", bufs=2) as ph5st, \
             tc.tile_pool(name="c1wp", bufs=1) as c1wp, \
             tc.tile_pool(name="ph5ps", bufs=1, space="PSUM") as ph5ps, \
             tc.tile_pool(name="c2psp", bufs=2, space="PSUM") as c2psp, \
             tc.tile_pool(name="ph5o", bufs=2) as ph5o:
            ctxQ = ph5.tile([128, NQT, 2 * H], BF16)
            decQ = ph5.tile([128, NQT, H], BF16)
            nc.vector.memset(ctxQ[:, :, :], 0.0)
            nc.vector.memset(decQ[:, :, :], 0.0)
            for q in range(NQT):
                rows = min(32, T - 32 * q) * Bl
                nc.sync.dma_start(
                    out=ctxQ[:rows, q, :],
                    in_=ctx_d[32 * q:32 * q + rows // Bl].flatten_outer_dims())
                nc.sync.dma_start(
                    out=decQ[:rows, q, :],
                    in_=dec_d[32 * q:32 * q + rows // Bl].flatten_outer_dims())
            ctxT = ph5.tile([128, H2K, 128 * NQT], BF16)
            decT = ph5.tile([128, HK, 128 * NQT], BF16)
            for q in range(NQT):
                pT = ph5ps.tile([128, H2K, 128], BF16, space="PSUM", tag="tps")
                for kc in range(H2K):
                    nc.tensor.transpose(out=pT[:, kc, :],
                                        in_=ctxQ[:, q, 128 * kc:128 * (kc + 1)],
                                        identity=I128b[:, :])
                nc.vector.tensor_copy(ctxT[:, :, 128 * q:128 * (q + 1)], pT[:, :, :])
                pT2 = ph5ps.tile([128, HK, 128], BF16, space="PSUM", tag="tps2")
                for kc in range(HK):
                    nc.tensor.transpose(out=pT2[:, kc, :],
                                        in_=decQ[:, q, 128 * kc:128 * (kc + 1)],
                                        identity=I128b[:, :])
                nc.vector.tensor_copy(decT[:, :, 128 * q:128 * (q + 1)], pT2[:, :, :])

            cls1b_s = ph5.tile([1, 1024], BF16)
            nc.gpsimd.dma_start(out=cls1b_s[:, :], in_=cls1b[:, :])
            h1Q = ph5.tile([128, NQT, 1024], BF16)
            lhs_chunks = ([(tembT, kc) for kc in range(EK)]
                          + [(ctxT, kc) for kc in range(H2K)]
                          + [(decT, kc) for kc in range(HK)])
            for n in range(2):
                c1w = c1wp.tile([128, G // 128, 512], BF16, tag="c1w")
                nc.gpsimd.dma_start(out=c1w[:, :, :],
                    in_=cls1WT[:, 512 * n:512 * (n + 1)].rearrange(
                        "(k p) n2 -> p k n2", p=128))
                for Tt in range(NQT):
                    ps = ph5ps.tile([128, 512], F32, space="PSUM", tag="c1ps")
                    for kg, (lt, kc) in enumerate(lhs_chunks):
                        nc.tensor.matmul(
                            out=ps[:, :],
                            lhsT=(lt[:, kc, 128 * Tt:128 * (Tt + 1)]),
                            rhs=(c1w[:, kg, :]),
                            start=(kg == 0), stop=False)
                    nc.tensor.matmul(out=ps[:, :], lhsT=(ones[:1, :128]),
                                     rhs=(cls1b_s[:, 512 * n:512 * (n + 1)]),
                                     start=False, stop=True)
                    nc.scalar.activation(out=h1Q[:, Tt, 512 * n:512 * (n + 1)],
                                         in_=ps[:, :], func=AF.Relu)
            h1T = ph5.tile([128, 8, 128 * NQT], BF16)
            for q in range(NQT):
                pT = ph5ps.tile([128, 8, 128], BF16, space="PSUM", tag="tps3")
                for kc in range(8):
                    nc.tensor.transpose(out=pT[:, kc, :],
                                        in_=h1Q[:, q, 128 * kc:128 * (kc + 1)],
                                        identity=I128b[:, :])
                nc.vector.tensor_copy(h1T[:, :, 128 * q:128 * (q + 1)], pT[:, :, :])

            def cls2_chunk(nv, nw):
                Wc = ph5st.tile([128, 8, 512], BF16, tag="c2w")
                # split the 1MB weight load across two DMA queues
                nc.gpsimd.dma_start(
                    out=Wc[:, 0:4, :nw],
                    in_=cls2WT[0:512, ds(nv, nw)].rearrange(
                        "(k p) n -> p k n", p=128))
                nc.scalar.dma_start(
                    out=Wc[:, 4:8, :nw],
                    in_=cls2WT[512:1024, ds(nv, nw)].rearrange(
                        "(k p) n -> p k n", p=128))
                bc_ = ph5st.tile([128, 512], BF16, tag="c2b")
                nc.gpsimd.dma_start(out=bc_[:, :nw], in_=cls2bR[:, ds(nv, nw)])
                for Tt in range(NQT):
                    trows = min(32, T - 32 * Tt)
                    ps = c2psp.tile([128, 512], F32, space="PSUM", tag="c2ps")
                    for kc in range(8):
                        nc.tensor.matmul(
                            out=ps[:, :nw],
                            lhsT=(h1T[:, kc, 128 * Tt:128 * (Tt + 1)]),
                            rhs=(Wc[:, kc, :nw]), start=(kc == 0), stop=(kc == 7))
                    ost = ph5o.tile([128, 512], BF16, tag="ost")
                    nc.vector.tensor_tensor(out=ost[:, :nw], in0=ps[:, :nw],
                                            in1=bc_[:, :nw], op=ALU.add)
                    nc.sync.dma_start(
                        out=logits[:, :, :].flatten_outer_dims()[
                            128 * Tt:128 * Tt + trows * Bl, ds(nv, nw)],
                        in_=ost[:trows * Bl, :nw])

            nfull = V // 512
            nd = (nfull // 4) * 4
            if nd > 0:
                with tc.For_i(0, 512 * nd, 2048) as nv0:
                    for uu in range(4):
                        cls2_chunk(nv0 + 512 * uu, 512)
            for start in range(512 * nd, V, 512):
                cls2_chunk(start, min(512, V - start))

        _stack.close()
    return nc


def prep_shared(p, V):
    """Core-independent input prep. p: dict of full-model params."""
    d = {}
    d["src_emb"] = np.ascontiguousarray(p["src_emb"], np.float32)
    d["tgt_emb"] = np.ascontiguousarray(p["tgt_emb"], np.float32)
    d["WihfT"] = np.ascontiguousarray(p["enc_Wih_f"].T).astype(BF)
    d["WihbT"] = np.ascontiguousarray(p["enc_Wih_b"].T).astype(BF)
    d["WhhfT"] = np.ascontiguousarray(p["enc_Whh_f"].T).astype(BF)
    d["WhhbT"] = np.ascontiguousarray(p["enc_Whh_b"].T).astype(BF)
    d["biasf"] = (p["enc_bih_f"] + p["enc_bhh_f"]).astype(BF)[None, :]
    d["biasb"] = (p["enc_bih_b"] + p["enc_bhh_b"]).astype(BF)[None, :]
    d["WiheT"] = np.ascontiguousarray(p["dec_Wih"][:, :E].T).astype(BF)
    d["WihcT"] = np.ascontiguousarray(p["dec_Wih"][:, E:].T).astype(BF)
    d["biasd"] = (p["dec_bih"] + p["dec_bhh"]).astype(BF)[None, :]
    # attention: fold |a2| into A1 rows (permuted pos-first); fold 0.5
    # (decoder 2x h-domain) into the h-contracting weights
    a2 = np.asarray(p["att2_W"][0], np.float32)
    perm = np.argsort(~(a2 > 0), kind="stable")     # positives first
    npos = int((a2 > 0).sum())
    sc = np.abs(a2[perm])[:, None]
    A1p = np.asarray(p["att1_W"], np.float32)[perm]
    d["A1eT"] = np.ascontiguousarray((A1p[:, :2 * H] * sc).T).astype(BF)
    d["att1b"] = (np.asarray(p["att1_b"], np.float32)[perm] * sc[:, 0]
                  ).astype(BF)[None, :]
    d["WqWhhT"] = np.ascontiguousarray(
        np.concatenate([(A1p[:, 2 * H:] * sc).T * 0.5,
                        p["dec_Whh"].T * 0.5], axis=1)).astype(BF)
    d["_npos"] = npos
    cls1 = np.asarray(p["cls1_W"], np.float32).copy()
    cls1[:, E + 2 * H:] *= 0.5          # decoder h stored as 2h
    d["cls1WT"] = np.ascontiguousarray(cls1.T).astype(BF)
    d["cls1b"] = p["cls1_b"].astype(BF)[None, :]
    d["cls2WT"] = np.ascontiguousarray(p["cls2_W"].T).astype(BF)
    d["cls2bR"] = np.broadcast_to(p["cls2_b"].astype(BF)[None, :],
                                  (128, V)).copy()
    mask4 = np.zeros((128, Bl), np.float32)
    for pp in range(128):
        mask4[pp, pp % Bl] = 1.0
    d["mask4"] = mask4
    S = 100
    NQ = (S + 31) // 32
    valid3 = (np.arange(128) // Bl + 32 * (NQ - 1)) < S
    d["mask4v"] = mask4 * valid3[:, None].astype(np.float32)
    m4F = np.concatenate([mask4] * (NQ - 1) + [d["mask4v"]], axis=1)
    d["mask4F"] = np.ascontiguousarray(m4F, np.float32)
    rep4 = np.zeros((Bl, 128), np.float32)
    for pp in range(128):
        rep4[pp % Bl, pp] = 1.0
    d["rep4"] = rep4
    return d


def idx_tile(tok, S):
    """tok: [Bl, S] int array -> [128, NQ] int32, token p=4*s'+b."""
    NQ = (S + 31) // 32
    out = np.zeros((128, NQ), np.int32)
    for q in range(NQ):
        for pp in range(128):
            b, sp = pp % Bl, pp // Bl
            s = 32 * q + sp
            if s < S:
                out[pp, q] = tok[b, s]
    return out


def vmask_tile(S):
    NQ = (S + 31) // 32
    out = np.zeros((128, NQ), np.float32)
    for q in range(NQ):
        for pp in range(128):
            if 32 * q + pp // Bl < S:
                out[pp, q] = 1.0
    return out


def prep_core(shared, source_data, target_data, core, S, T):
    d = {k: v for k, v in shared.items() if not k.startswith("_")}
    d["idx_src"] = idx_tile(source_data[4 * core:4 * core + 4], S)
    d["idx_tgt"] = idx_tile(target_data[4 * core:4 * core + 4], T)
    return d


def np_reference(src, tgt, p):
    """Port of reference.py for batch rows in src/tgt [B, S]."""
    def sig(x): return 1.0 / (1.0 + np.exp(-x))

    def lstm_step(x, h, c, Wih, Whh, bih, bhh):
        g = x @ Wih.T + h @ Whh.T + (bih + bhh)
        i, f, gg, o = np.split(g, 4, axis=-1)
        c = sig(f) * c + sig(i) * np.tanh(gg)
        h = sig(o) * np.tanh(c)
        return h, c

    B, S = src.shape
    T = tgt.shape[1]
    x = p["src_emb"][src].transpose(1, 0, 2).astype(np.float32)
    z = np.zeros((B, H), np.float32)
    hf, cf = z, z
    of = []
    for s in range(S):
        hf, cf = lstm_step(x[s], hf, cf, p["enc_Wih_f"], p["enc_Whh_f"],
                           p["enc_bih_f"], p["enc_bhh_f"])
        of.append(hf)
    hb, cb = z, z
    ob = []
    for s in range(S):
        hb, cb = lstm_step(x[S - 1 - s], hb, cb, p["enc_Wih_b"], p["enc_Whh_b"],
                           p["enc_bih_b"], p["enc_bhh_b"])
        ob.append(hb)
    of = np.stack(of); ob = np.stack(ob)
    enc = np.concatenate([of, ob[::-1]], -1).transpose(1, 0, 2)  # [B,S,2H]
    temb = p["tgt_emb"][tgt].astype(np.float32)                  # [B,T,E]
    h, c = hf, cf
    ctxs, decs = [], []
    for t in range(T):
        prev = np.broadcast_to(h[:, None, :], (B, S, H))
        ain = np.concatenate([enc, prev], -1)
        hid = np.maximum(ain @ p["att1_W"].T + p["att1_b"], 0.0)
        sc = hid @ p["att2_W"].T + p["att2_b"]
        w = np.exp(sc - sc.max(axis=1, keepdims=True))
        w = w / w.sum(axis=1, keepdims=True)
        ctx = (w * enc).sum(axis=1)
        h, c = lstm_step(np.concatenate([temb[:, t], ctx], -1), h, c,
                         p["dec_Wih"], p["dec_Whh"], p["dec_bih"], p["dec_bhh"])
        ctxs.append(ctx); decs.append(h)
    ctxs = np.stack(ctxs, 1); decs = np.stack(decs, 1)
    ci = np.concatenate([temb, ctxs, decs], -1)
    h1 = np.maximum(ci @ p["cls1_W"].T + p["cls1_b"], 0.0)
    return h1 @ p["cls2_W"].T + p["cls2_b"]

# ===================== host-side entry point =====================
_CACHE = {}


def _get_nc(npos):
    if "nc" not in _CACHE:
        nc = build_nc(S=100, T=100, V=32000, num_devices=8, unroll=5,
                      npos=npos)
        nc.compile()
        _CACHE["nc"] = nc
    return _CACHE["nc"]


def kernel(trace=False, **inputs):
    S = T = 100
    V = 32000
    B = 32
    from concourse.bass_utils import run_bass_kernel_spmd
    shared = prep_shared(inputs, V)
    nc = _get_nc(shared["_npos"])
    src = np.asarray(inputs["source_data"])
    tgt = np.asarray(inputs["target_data"])
    in_maps = [prep_core(shared, src, tgt, c, S, T) for c in range(8)]
    res = run_bass_kernel_spmd(nc, in_maps, core_ids=list(range(8)),
                               trace=trace)
    out = np.empty((B, T, V), np.float32)
    for c in range(8):
        lg = np.asarray(res.results[c]["logits"]).astype(
            np.float32).reshape(T, Bl, V)
        out[4 * c:4 * c + 4] = lg.transpose(1, 0, 2)
    if trace:
        _CACHE["exec_time_ns"] = res.exec_time_ns
        _CACHE["profile"] = res
    return out
